# revision 35
# baseline (speedup 1.0000x reference)
"""Trainium2 Bass kernel for a 6-layer transformer encoder.

Model (from the reference): B=32, S=512, D=512, H=8 heads (DK=64), FFN=2048,
V=32000, 6 layers, post-LN, no FFN activation, eval mode (dropout=identity).

Sharding: data-parallel over batch. Each of the 8 NeuronCores processes 4
sequences (2048 tokens) end-to-end; weights are replicated. No collectives.

Device-side layout strategy (per core):
  - Residual stream x kept natural [token(part), feature(free)] as two
    [128, 16, 512] fp32 SBUF tiles (token t = tt*128 + p); LayerNorm
    applies in place so only two buffers rotate.
  - Matmul operands are produced transposed ([feature, token]) via
    DMA-xbar transposes of a bf16 copy of the stream (the bf16 copy is
    emitted by the LayerNorm apply itself); matmuls are bf16 with fp32
    PSUM accumulation, N=512, fully K-contiguous so the PE stays warm.
  - Attention scores are computed transposed (energyT[s, q]) so softmax
    runs over the free axis; per head-pair the two K=64 score matmuls
    are issued adjacently at partition bases 0/64 so the PE row-tiles
    them concurrently. Softmax denominators come free from a ones-row
    appended to V ("Vaug"); exp(x) needs no max-subtraction (|scores|
    are small by construction). Normalization is deferred: 1/denom
    (reciprocal_approx) is broadcast across partitions with K=1 PE
    outer-products and applied to the collected head outputs.
  - The attention b-loop runs with a 1-lag software pipeline: sequence
    b+1's transposes/QKV/scores overlap sequence b's normalize+proj.
  - LayerNorm stats via bn_stats/bn_aggr (free-dim reduction); apply is
    one per-partition-affine ACT op (scale=1/sd, bias=-mu/sd).
  - Linear biases are folded in as K=1 matmuls into the same PSUM
    accumulation group (bo, b2) or as a per-partition ACT bias (b1).
"""

import sys

sys.path.insert(0, "/opt/trn_rl_repo")

import numpy as np
import ml_dtypes

import concourse.bass as bass
import concourse.tile as tile
from concourse import bacc, mybir
from concourse.bass_utils import run_bass_kernel_spmd

AF = mybir.ActivationFunctionType
ALU = mybir.AluOpType
DT = mybir.dt
F32 = DT.float32
BF16 = DT.bfloat16

# Model dims (hardcoded per the problem spec).
V, L, H, D, FE, S, B = 32000, 6, 8, 512, 4, 512, 32
DK = D // H          # 64
FD = FE * D          # 2048
NCORES = 8
BL = B // NCORES     # 4 sequences per core
T = BL * S           # 2048 tokens per core
P = 128
NTT = T // P         # 16 token tiles per core
EPS = 1e-5

_CACHE: dict = {}


def _pos_encoding_np(seq_len: int, d: int) -> np.ndarray:
    pos = np.arange(seq_len, dtype=np.float32)[:, None]
    div = np.exp(np.arange(0, d, 2, dtype=np.float32) * (-np.log(10000.0) / d))
    pe = np.zeros((seq_len, d), dtype=np.float32)
    pe[:, 0::2] = np.sin(pos * div)
    pe[:, 1::2] = np.cos(pos * div)
    return pe


def _build_nc(num_layers: int, with_affine: bool):
    nc = bacc.Bacc("TRN2", target_bir_lowering=False, debug=False,
                   num_devices=NCORES)

    tokens_d = nc.dram_tensor("tokens", [T], DT.int32, kind="ExternalInput")
    emb_d = nc.dram_tensor("emb", [V, D], F32, kind="ExternalInput")
    pe_d = nc.dram_tensor("pe", [S, D], F32, kind="ExternalInput")
    wq_d = nc.dram_tensor("wq", [L, D, D], BF16, kind="ExternalInput")
    wk_d = nc.dram_tensor("wk", [L, D, D], BF16, kind="ExternalInput")
    wv_d = nc.dram_tensor("wv", [L, D, D], BF16, kind="ExternalInput")
    wo_d = nc.dram_tensor("wo", [L, D, D], BF16, kind="ExternalInput")
    w1_d = nc.dram_tensor("w1", [L, D, FD], BF16, kind="ExternalInput")
    w2_d = nc.dram_tensor("w2", [L, FD, D], BF16, kind="ExternalInput")
    bo_d = nc.dram_tensor("bo", [L, D], BF16, kind="ExternalInput")
    b1_d = nc.dram_tensor("b1", [L, FD], F32, kind="ExternalInput")
    b2_d = nc.dram_tensor("b2", [L, D], BF16, kind="ExternalInput")
    if with_affine:
        g1_d = nc.dram_tensor("g1", [L, D], F32, kind="ExternalInput")
        be1_d = nc.dram_tensor("be1", [L, D], F32, kind="ExternalInput")
        g2_d = nc.dram_tensor("g2", [L, D], F32, kind="ExternalInput")
        be2_d = nc.dram_tensor("be2", [L, D], F32, kind="ExternalInput")
    out_d = nc.dram_tensor("out", [T, D], F32, kind="ExternalOutput")

    from contextlib import ExitStack

    with tile.TileContext(nc) as tc, ExitStack() as ctx:
        const = ctx.enter_context(tc.tile_pool(name="const", bufs=1))
        xpool = ctx.enter_context(tc.tile_pool(name="x", bufs=1))
        wts = ctx.enter_context(tc.tile_pool(name="wts", bufs=1))
        acts = ctx.enter_context(tc.tile_pool(name="acts", bufs=2))
        small = ctx.enter_context(tc.tile_pool(name="small", bufs=4))
        ffn = ctx.enter_context(tc.tile_pool(name="ffn", bufs=1))
        den = ctx.enter_context(tc.tile_pool(name="den", bufs=2))
        aff = ctx.enter_context(tc.tile_pool(name="aff", bufs=1)) if with_affine else None
        pmm = ctx.enter_context(tc.tile_pool(name="pmm", bufs=7, space="PSUM"))
        prb = ctx.enter_context(tc.tile_pool(name="prb", bufs=1, space="PSUM"))

        ones_bf = const.tile([1, P], BF16)
        nc.gpsimd.memset(ones_bf[:], 1.0)
        ones_f = const.tile([P, 64], F32)
        nc.gpsimd.memset(ones_f[:], 1.0)
        eps_sb = const.tile([P, 1], F32)
        nc.gpsimd.memset(eps_sb[:], EPS)

        idx_sb = const.tile([P, NTT], DT.int32)
        nc.sync.dma_start(idx_sb[:], tokens_d.ap().rearrange("(a p) -> p a", p=P))

        # Two fp32 residual-stream buffers, roles alternate within a layer.
        xA = xpool.tile([P, NTT, D], F32)
        xB = xpool.tile([P, NTT, D], F32)

        # ---- embedding gather + positional encoding ----
        xbf_cur = ffn.tile([P, NTT, D], BF16, tag="xbf")
        pe_r = pe_d.ap().rearrange("(j p) d -> p j d", p=P)
        for tt in range(NTT):
            gx = ffn.tile([P, D], F32, tag="hT")
            nc.gpsimd.indirect_dma_start(
                out=gx[:], out_offset=None, in_=emb_d.ap(),
                in_offset=bass.IndirectOffsetOnAxis(ap=idx_sb[:, tt:tt + 1], axis=0),
            )
            nc.sync.dma_start(xA[:, tt, :], pe_r[:, tt % 4, :])
            nc.vector.tensor_tensor(out=xA[:, tt, :], in0=xA[:, tt, :],
                                    in1=gx[:], op=ALU.add)
            nc.vector.tensor_copy(xbf_cur[:, tt, :], xA[:, tt, :])

        def layer_norm_inplace(xt, l, ln_par):
            g_sb = b_sb = None
            if ln_par is not None:
                g_d_, b_d_ = ln_par
                g_sb = wts.tile([P, D], F32, tag="gg")
                nc.sync.dma_start(g_sb[:], g_d_.ap()[l][None, :].to_broadcast([P, D]))
                b_sb = wts.tile([P, D], F32, tag="bb")
                nc.sync.dma_start(b_sb[:], b_d_.ap()[l][None, :].to_broadcast([P, D]))
            # normalizes xt in place (fp32) and also emits a bf16 copy for
            # the following matmul phase; returns the bf16 tile
            xbf = ffn.tile([P, NTT, D], BF16, tag="xbf")
            for tt in range(NTT):
                st6 = small.tile([P, 6], F32, tag="st6")
                nc.vector.bn_stats(st6[:], xt[:, tt, :])
                mv = small.tile([P, 2], F32, tag="mv")
                nc.vector.bn_aggr(mv[:], st6[:])
                sd = small.tile([P, 1], F32, tag="sd")
                nc.scalar.activation(sd[:], mv[:, 1:2], AF.Sqrt, bias=eps_sb[:])
                rs = small.tile([P, 1], F32, tag="rs")
                nc.vector.reciprocal_approx_fast(out=rs[:], in_=sd[:])
                nmurs = small.tile([P, 1], F32, tag="nmurs")
                nc.vector.tensor_tensor(out=nmurs[:], in0=mv[:, 0:1],
                                        in1=rs[:], op=ALU.mult)
                nc.vector.tensor_scalar(out=nmurs[:], in0=nmurs[:],
                                        scalar1=-1.0, scalar2=None,
                                        op0=ALU.mult)
                nc.scalar.activation(xt[:, tt, :], xt[:, tt, :], AF.Identity,
                                     bias=nmurs[:], scale=rs[:])
                if with_affine:
                    nc.vector.tensor_tensor(out=xt[:, tt, :], in0=xt[:, tt, :],
                                            in1=g_sb[:], op=ALU.mult)
                    nc.vector.tensor_tensor(out=xt[:, tt, :], in0=xt[:, tt, :],
                                            in1=b_sb[:], op=ALU.add)
                    nc.vector.tensor_copy(xbf[:, tt, :], xt[:, tt, :])
                else:
                    nc.vector.tensor_copy(xbf[:, tt, :], xt[:, tt, :])
            return xbf

        x_in, x_alt = xA, xB
        for l in range(num_layers):
            # ---- per-layer weights ----
            wq_sb = wts.tile([P, 4, D], BF16, tag="wq")
            nc.sync.dma_start(wq_sb[:], wq_d.ap()[l].rearrange("(dc p) f -> p dc f", p=P))
            wk_sb = wts.tile([P, 4, D], BF16, tag="wk")
            nc.sync.dma_start(wk_sb[:], wk_d.ap()[l].rearrange("(dc p) f -> p dc f", p=P))
            wv_sb = wts.tile([P, 4, D], BF16, tag="wv")
            nc.sync.dma_start(wv_sb[:], wv_d.ap()[l].rearrange("(dc p) f -> p dc f", p=P))
            wo_sb = wts.tile([P, 4, D], BF16, tag="wo")
            nc.sync.dma_start(wo_sb[:], wo_d.ap()[l].rearrange("(dc p) f -> p dc f", p=P))
            w1_sb = wts.tile([P, 4, FD], BF16, tag="w1")
            nc.sync.dma_start(w1_sb[:], w1_d.ap()[l].rearrange("(dc p) f -> p dc f", p=P))
            w2_sb = wts.tile([P, 16, D], BF16, tag="w2")
            nc.sync.dma_start(w2_sb[:], w2_d.ap()[l].rearrange("(hc p) f -> p hc f", p=P))
            bo_sb = wts.tile([1, D], BF16, tag="bo")
            nc.sync.dma_start(bo_sb[:], bo_d.ap()[l][None, :])
            b1_sb = wts.tile([P, 16], F32, tag="b1")
            nc.sync.dma_start(b1_sb[:], b1_d.ap()[l].rearrange("(ft p) -> p ft", p=P))
            b2_sb = wts.tile([1, D], BF16, tag="b2")
            nc.sync.dma_start(b2_sb[:], b2_d.ap()[l][None, :])
            ln1_par = (g1_d, be1_d) if with_affine else None
            ln2_par = (g2_d, be2_d) if with_affine else None

            # ---- attention, per sequence b (1-lag software pipeline) ----
            xbf = xbf_cur

            def finish_b(headU, dn_b, b):
                # normalize: headU[h] *= (1/denom[q]) broadcast over partitions
                nc.vector.reciprocal_approx_fast(out=dn_b[:], in_=dn_b[:])
                for hp in range(4):
                    rb = prb.tile([P, S], F32, tag="rb")
                    for i in range(2):
                        h = 2 * hp + i
                        pb = 32 * (h % 3)
                        nc.tensor.matmul(rb[64 * i:64 * i + 64, :],
                                         ones_f[pb:pb + 1, :],
                                         dn_b[pb:pb + 1, h // 3, :],
                                         start=True, stop=True)
                    nc.vector.tensor_tensor(out=headU[:, hp, :],
                                            in0=headU[:, hp, :],
                                            in1=rb[:], op=ALU.mult)
                # output projection + bias + residual
                for j in range(4):
                    pa = pmm.tile([P, 512], F32, tag="mm")
                    for dc in range(4):
                        nc.tensor.matmul(pa[:], headU[:, dc, j * P:(j + 1) * P],
                                         wo_sb[:, dc, :],
                                         start=(dc == 0), stop=False)
                    nc.tensor.matmul(pa[:], ones_bf[:], bo_sb[:],
                                     start=False, stop=True)
                    tt = b * 4 + j
                    nc.vector.tensor_tensor(out=x_alt[:, tt, :], in0=pa[:],
                                            in1=x_in[:, tt, :], op=ALU.add)

            pending = None
            for b in range(BL):
                # transpose this sequence's activations: xT_b[p, dc, t] (bf16)
                xT_b = acts.tile([P, 4, S], BF16, tag="xT")
                for j in range(4):
                    nc.sync.dma_start_transpose(
                        out=xT_b[:, :, j * P:(j + 1) * P],
                        in_=xbf[:, b * 4 + j, :])

                # Q^T, K^T: [hk(part), t] ; 2 heads per 128-partition tile
                qt = acts.tile([P, 4, S], BF16, tag="qt")
                kt = acts.tile([P, 4, S], BF16, tag="kt")
                for w_sb, dst in ((wq_sb, qt), (wk_sb, kt)):
                    for hp in range(4):
                        pq = pmm.tile([P, 512], F32, tag="mm")
                        for dc in range(4):
                            nc.tensor.matmul(
                                pq[:], w_sb[:, dc, hp * P:(hp + 1) * P],
                                xT_b[:, dc, :], start=(dc == 0), stop=(dc == 3))
                        nc.vector.tensor_copy(dst[:, hp, :], pq[:])

                # V natural [t(part), h, dk] augmented with a ones column
                vaug = ffn.tile([P, 4, H, DK + 1], BF16, tag="vaug")
                nc.gpsimd.memset(vaug[:, :, :, DK:DK + 1], 1.0)
                for j in range(4):
                    pv = pmm.tile([P, 512], F32, tag="mm")
                    for dc in range(4):
                        nc.tensor.matmul(
                            pv[:], xT_b[:, dc, j * P:(j + 1) * P],
                            wv_sb[:, dc, :], start=(dc == 0), stop=(dc == 3))
                    nc.vector.tensor_copy(vaug[:, j, :, 0:DK],
                                          pv[:].rearrange("p (h k) -> p h k", h=H))

                # scores (transposed) -> exp -> A@V with ones-row denominators
                headU = acts.tile([P, 4, S], BF16, tag="hU")
                # per-head softmax denominators parked at the 3 matmul-legal
                # partition bases (0/32/64) x 3 free slots
                dn_b = den.tile([P, 3, S], F32, tag="dn")
                for hp in range(4):
                    # both heads of the pair issue adjacent K=64 matmuls at
                    # partition bases 0 and 64 -> concurrent row-tiles on PE
                    expa = (aff or den).tile([P, 4, S], BF16, tag="exps")
                    expb = ffn.tile([P, 4, S], BF16, tag="expsb")
                    for st in range(4):
                        pea = pmm.tile([P, 512], F32, tag="mm")
                        peb = pmm.tile([P, 512], F32, tag="mm")
                        nc.tensor.matmul(
                            pea[:], kt[0:64, hp, st * P:(st + 1) * P],
                            qt[0:64, hp, :], start=True, stop=True)
                        nc.tensor.matmul(
                            peb[:], kt[64:128, hp, st * P:(st + 1) * P],
                            qt[64:128, hp, :], start=True, stop=True)
                        nc.scalar.activation(expa[:, st, :], pea[:], AF.Exp,
                                             scale=0.125)
                        nc.scalar.activation(expb[:, st, :], peb[:], AF.Exp,
                                             scale=0.125)
                    for i, exps in ((0, expa), (1, expb)):
                        h = 2 * hp + i
                        ph = pmm.tile([DK + 1, 512], F32, tag="mm")
                        for st in range(4):
                            nc.tensor.matmul(ph[:], vaug[:, st, h, :],
                                             exps[:, st, :],
                                             start=(st == 0), stop=(st == 3))
                        pb = 32 * (h % 3)
                        nc.scalar.copy(dn_b[pb:pb + 1, h // 3, :],
                                       ph[DK:DK + 1, :])
                        nc.vector.tensor_copy(headU[64 * i:64 * i + 64, hp, :],
                                              ph[0:DK, :])

                if pending is not None:
                    finish_b(*pending)
                pending = (headU, dn_b, b)
            finish_b(*pending)

            xbf2 = layer_norm_inplace(x_alt, l, ln1_par)

            # ---- FFN, per 512-token chunk ----
            for tc_i in range(4):
                xT2 = acts.tile([P, 4, S], BF16, tag="xT")
                for j in range(4):
                    nc.sync.dma_start_transpose(
                        out=xT2[:, :, j * P:(j + 1) * P],
                        in_=xbf2[:, tc_i * 4 + j, :])
                hT = ffn.tile([P, 16, S], BF16, tag="hT")
                for ft in range(16):
                    ph1 = pmm.tile([P, 512], F32, tag="mm")
                    for dc in range(4):
                        nc.tensor.matmul(ph1[:], w1_sb[:, dc, ft * P:(ft + 1) * P],
                                         xT2[:, dc, :],
                                         start=(dc == 0), stop=(dc == 3))
                    nc.scalar.activation(hT[:, ft, :], ph1[:], AF.Identity,
                                         bias=b1_sb[:, ft:ft + 1])
                for j in range(4):
                    pf = pmm.tile([P, 512], F32, tag="mm")
                    for hc in range(16):
                        nc.tensor.matmul(pf[:], hT[:, hc, j * P:(j + 1) * P],
                                         w2_sb[:, hc, :],
                                         start=(hc == 0), stop=False)
                    nc.tensor.matmul(pf[:], ones_bf[:], b2_sb[:],
                                     start=False, stop=True)
                    tt = tc_i * 4 + j
                    nc.vector.tensor_tensor(out=x_in[:, tt, :], in0=pf[:],
                                            in1=x_alt[:, tt, :], op=ALU.add)

            xbf_cur = layer_norm_inplace(x_in, l, ln2_par)
            # x_in now holds the layer output; buffers keep their roles.

        nc.sync.dma_start(out_d.ap().rearrange("(tt p) f -> p tt f", p=P),
                          x_in[:])

    nc.compile()
    return nc


def _prep_weights(inputs: dict) -> dict:
    """Host-side cast + layout of the replicated weights (shared by cores)."""
    bf = ml_dtypes.bfloat16
    wq = np.ascontiguousarray(inputs["Wq"].transpose(0, 2, 1, 3)
                              .reshape(L, D, D)).astype(bf)
    wk = np.ascontiguousarray(inputs["Wk"].transpose(0, 2, 1, 3)
                              .reshape(L, D, D)).astype(bf)
    wv = np.ascontiguousarray(inputs["Wv"].transpose(0, 2, 1, 3)
                              .reshape(L, D, D)).astype(bf)
    return {
        "emb": np.ascontiguousarray(inputs["emb"], dtype=np.float32),
        "pe": _pos_encoding_np(S, D),
        "wq": wq, "wk": wk, "wv": wv,
        "wo": np.ascontiguousarray(inputs["Wo"]).astype(bf),
        "w1": np.ascontiguousarray(inputs["W1"]).astype(bf),
        "w2": np.ascontiguousarray(inputs["W2"]).astype(bf),
        "bo": np.ascontiguousarray(inputs["bo"]).astype(bf),
        "b1": np.ascontiguousarray(inputs["b1"], dtype=np.float32),
        "b2": np.ascontiguousarray(inputs["b2"]).astype(bf),
    }


def kernel(num_layers: int = L, **inputs) -> np.ndarray:
    tokens = np.asarray(inputs["tokens"])
    assert tokens.shape == (B, S) and tokens.dtype == np.int32

    g_ones = (np.all(inputs["ln1_g"] == 1.0) and np.all(inputs["ln2_g"] == 1.0)
              and np.all(inputs["ln1_b"] == 0.0) and np.all(inputs["ln2_b"] == 0.0))
    with_affine = not g_ones

    key = (num_layers, with_affine)
    if key not in _CACHE:
        _CACHE[key] = _build_nc(num_layers, with_affine)
    nc = _CACHE[key]

    shared = _prep_weights(inputs)
    if with_affine:
        shared = dict(shared)
        shared["g1"] = np.ascontiguousarray(inputs["ln1_g"], dtype=np.float32)
        shared["be1"] = np.ascontiguousarray(inputs["ln1_b"], dtype=np.float32)
        shared["g2"] = np.ascontiguousarray(inputs["ln2_g"], dtype=np.float32)
        shared["be2"] = np.ascontiguousarray(inputs["ln2_b"], dtype=np.float32)

    in_maps = []
    for c in range(NCORES):
        m = dict(shared)
        m["tokens"] = np.ascontiguousarray(
            tokens[c * BL:(c + 1) * BL].reshape(T))
        in_maps.append(m)

    res = run_bass_kernel_spmd(nc, in_maps, core_ids=list(range(NCORES)))
    out = np.concatenate(
        [res.results[c]["out"].reshape(BL, S, D) for c in range(NCORES)], axis=0)
    return out
